# revision 69
# baseline (speedup 1.0000x reference)
"""Trainium2 Bass kernel for nn_AttentionUnit (multi-head attention block), v2.

Reference math (B=2, S=2048, D=1024, H=16 heads, d_head=64, fp32):
    Q = q @ wq.T + bq ; K = k @ wk.T + bk ; V = v @ wv.T + bv
    S = QK^T / 8  (per head), causal mask + key-padding mask
    out = softmax(S) @ V  -> concat heads -> @ wo.T + bo

Sharding (8 cores): data-parallel over batch (2 groups of 4 cores),
tensor-parallel over heads (4 heads/core).  Column-parallel QKV,
row-parallel wo; partial outputs are summed with in-group
ReduceScatters (per q block for qb0/qb1, one merged collective for
qb2+qb3) that pipeline behind compute; the host reassembles the
row-range outputs.

Schedule: all emission units (projection blocks, score heads, PV pairs,
out-proj) are generators woven into one PE stream.  Score tiles are
exp-paced (one [128,512] Exp drains in ~610ns vs the 213ns score
matmul), so after each score tile ~3 steps of queued exp-free matmul
work are pumped into the stream; the scalar engine then runs the exp
stream back-to-back while the PE stays busy, and out-proj chunks finish
early enough that only the last collective is tail-exposed.

Other layout choices:
  - K/Q projections run contraction-outer over 4 PSUM banks, fed by
    consumption-ordered DMAs, so the PE starts ~1us into the kernel and
    the softmax Exp stream (the scalar-engine bottleneck) starts ~8us.
  - Q/K biases are applied during PSUM eviction (per-partition
    tensor_scalar add) instead of riding the matmul contraction; K2/Q2
    activations are kept mt-major [128, S] so eviction is one op/bank.
  - Scores are computed transposed (S^T[k, q]) per head; exp on the
    scalar engine; causal masking multiplies only the 128-wide diagonal
    sub-tile by a lower-triangle constant.  P^T chunks are the
    stationary operand of PV, so ctx lands q-major and the softmax
    division is a per-partition reciprocal+scale; head pairs share one
    [128,128] PE transpose into ctx^T for the output projection.
  - Engine placement: scalar = exp + small consts + final output
    bounce; DVE = all PSUM evictions, masks, ctx normalize chain;
    gpsimd = collectives + early-phase input DMAs; sync = bulk input
    DMAs + partial-output writes.
"""

import os
import sys
from collections import deque
from contextlib import ExitStack

import numpy as np

try:
    import concourse.bass as bass
except ImportError:  # harness containers keep the repo at /opt/trn_rl_repo
    for _p in ("/opt/trn_rl_repo", "/root/.axon_site/_ro/trn_rl_repo"):
        if os.path.isdir(_p) and _p not in sys.path:
            sys.path.insert(0, _p)
    import concourse.bass as bass

from concourse import bacc

import ml_dtypes
import concourse.mybir as mybir
import concourse.tile as tile
from concourse.bass_utils import run_bass_kernel_spmd

BF16 = ml_dtypes.bfloat16

B = 2
SEQ = 2048
D = 1024
H = 16
DH = 64
NCORES = 8
G = 4            # tensor-parallel group size (cores per batch)
HPC = H // G     # heads per core
DPC = HPC * DH   # head dims per core (256)
QB = 512         # q block width
KT = 128         # k tile height
NMT = DPC // 128  # mt tiles of per-core head dims (2)
NDT = D // 128    # contraction tiles of the model dim (8)
NQB = SEQ // QB   # q blocks (4)
SUB = QB // KT    # k tiles per q block on the diagonal (4)


def build_program(use_kpm=False):
    """Emit the SPMD program (identical on all 8 cores)."""
    fp32 = mybir.dt.float32
    bf16 = mybir.dt.bfloat16

    nc = bacc.Bacc(num_devices=NCORES)

    xqT = nc.declare_dram_parameter("xqT", [D, SEQ], bf16, False)
    xkT = nc.declare_dram_parameter("xkT", [D, SEQ], bf16, False)
    xvT = nc.declare_dram_parameter("xvT", [D, SEQ], bf16, False)
    wqT = nc.declare_dram_parameter("wqT", [D, DPC], bf16, False)
    wkT = nc.declare_dram_parameter("wkT", [D, DPC], bf16, False)
    wvT = nc.declare_dram_parameter("wvT", [D, DPC], bf16, False)
    wvb = nc.declare_dram_parameter("wvb", [1, DPC], bf16, False)
    woT = nc.declare_dram_parameter("woT", [DPC, D], bf16, False)
    bq2_d = nc.declare_dram_parameter("bq2", [128, NMT], fp32, False)
    bk2_d = nc.declare_dram_parameter("bk2", [128, NMT], fp32, False)
    tri_d = nc.declare_dram_parameter("tri", [KT, KT], bf16, False)
    ident_d = nc.declare_dram_parameter("ident", [128, 128], bf16, False)
    bo2_d = nc.declare_dram_parameter("bo2", [128, D], fp32, False)
    kpm_d = (nc.declare_dram_parameter("kpmT", [128, SEQ // 128], fp32, False)
             if use_kpm else None)
    out_ext = nc.declare_dram_parameter("out", [SEQ // G, D], bf16, isOutput=True)

    partial_dram = nc.dram_tensor("partial", [SEQ, D], bf16)
    rs_out = nc.dram_tensor("rs_out", [SEQ // G, D], bf16)

    groups = [[0, 1, 2, 3], [4, 5, 6, 7]]

    with ExitStack() as ctx:
        tc = ctx.enter_context(tile.TileContext(nc, num_cores=NCORES))

        xpool = ctx.enter_context(tc.tile_pool(name="xp", bufs=28))
        persist = ctx.enter_context(tc.tile_pool(name="persist", bufs=1))
        ppool = ctx.enter_context(tc.tile_pool(name="pp", bufs=34))
        cqpool = ctx.enter_context(tc.tile_pool(name="cq", bufs=4))
        opool = ctx.enter_context(tc.tile_pool(name="op", bufs=4))
        spool = ctx.enter_context(tc.tile_pool(name="sp", bufs=8))
        psP = ctx.enter_context(tc.tile_pool(name="psP", bufs=2, space="PSUM"))
        psM = ctx.enter_context(tc.tile_pool(name="psM", bufs=2, space="PSUM"))
        psC = ctx.enter_context(tc.tile_pool(name="psC", bufs=2, space="PSUM"))

        # ---- small constants (spread over queues; all early) ----
        tri_sb = persist.tile([KT, KT], bf16, tag="tri")
        nc.scalar.dma_start(out=tri_sb, in_=tri_d[:, :])
        ident_sb = persist.tile([128, 128], bf16, tag="ident")
        nc.scalar.dma_start(out=ident_sb, in_=ident_d[:, :])
        bq2_sb = persist.tile([128, NMT], fp32, tag="bq2")
        nc.scalar.dma_start(out=bq2_sb, in_=bq2_d[:, :])
        bk2_sb = persist.tile([128, NMT], fp32, tag="bk2")
        nc.scalar.dma_start(out=bk2_sb, in_=bk2_d[:, :])
        bo2_sb = persist.tile([128, D], fp32, tag="bo2")
        nc.scalar.dma_start(out=bo2_sb, in_=bo2_d[:, :])
        wvb_sb = persist.tile([1, DPC], bf16, tag="wvb")
        nc.scalar.dma_start(out=wvb_sb, in_=wvb[0:1, :])
        ones1 = persist.tile([1, 128], bf16, tag="ones1")
        nc.vector.memset(ones1, 1.0)
        kpm_sb = None
        if use_kpm:
            kpm_sb = persist.tile([128, SEQ // 128], fp32, tag="kpm")
            nc.scalar.dma_start(out=kpm_sb, in_=kpm_d[:, :])
        wo_sb = [persist.tile([128, D], bf16, tag=f"wo{t}", name=f"wo{t}")
                 for t in range(NMT)]
        for t in range(NMT):
            nc.scalar.dma_start(out=wo_sb[t], in_=woT[t * 128:(t + 1) * 128, :])

        # ---- persistent weights (consumed every block; load once) ----
        wk_t = [persist.tile([128, DPC], bf16, tag=f"wk{k}", name=f"wk{k}")
                for k in range(NDT)]
        wq_t = [persist.tile([128, DPC], bf16, tag=f"wq{k}", name=f"wq{k}")
                for k in range(NDT)]
        wv_t = [persist.tile([128, DPC], bf16, tag=f"wv{k}", name=f"wv{k}")
                for k in range(NDT)]

        # ---- persistent activations ----
        # K2/Q2 are mt-major: rows = the 128 head dims of heads (2mt, 2mt+1).
        K2 = [persist.tile([128, SEQ], bf16, tag=f"K2{t}", name=f"K2{t}")
              for t in range(NMT)]
        Q2 = [persist.tile([128, SEQ], bf16, tag=f"Q2{t}", name=f"Q2{t}")
              for t in range(NMT)]
        V_sb = [persist.tile([128, HPC, 65], bf16, tag=f"V{m}", name=f"V{m}")
                for m in range(SEQ // KT)]
        ctxT = [persist.tile([128, SEQ], bf16, tag=f"ctxT{t}", name=f"ctxT{t}")
                for t in range(NMT)]

        def kq_dma(blk):
            """Issue K/Q input DMAs for q/k columns [blk*QB, (blk+1)*QB)."""
            c0 = blk * QB
            eng = nc.gpsimd if blk <= 1 else nc.sync
            xk_t, xq_t = [], []
            for k in range(NDT):
                xk = xpool.tile([128, QB], bf16, tag="xt", name=f"xk{blk}_{k}")
                nc.sync.dma_start(out=xk, in_=xkT[k * 128:(k + 1) * 128, c0:c0 + QB])
                if blk == 0:
                    nc.sync.dma_start(out=wk_t[k], in_=wkT[k * 128:(k + 1) * 128, :])
                xq = xpool.tile([128, QB], bf16, tag="xt", name=f"xq{blk}_{k}")
                eng.dma_start(out=xq, in_=xqT[k * 128:(k + 1) * 128, c0:c0 + QB])
                if blk == 0:
                    eng.dma_start(out=wq_t[k], in_=wqT[k * 128:(k + 1) * 128, :])
                xk_t.append(xk)
                xq_t.append(xq)
            return xk_t, xq_t

        def kq_gen(blk, mt, xk_t, xq_t):
            """K and Q projection matmuls for one mt half (yield/matmul).
            Splitting per-mt lets scores for heads 2mt/2mt+1 start after
            only this half's eviction."""
            c0 = blk * QB
            pskq = psP.tile([128, 2 * QB], fp32, tag="p2", name="pskq")
            psk, psq = pskq[:, 0:QB], pskq[:, QB:2 * QB]
            for k in range(NDT):
                st, sp = (k == 0), (k == NDT - 1)
                nc.tensor.matmul(out=psk, rhs=xk_t[k],
                                 lhsT=wk_t[k][:, mt * 128:(mt + 1) * 128],
                                 start=st, stop=sp)
                yield
                nc.tensor.matmul(out=psq, rhs=xq_t[k],
                                 lhsT=wq_t[k][:, mt * 128:(mt + 1) * 128],
                                 start=st, stop=sp)
                yield
            nc.vector.tensor_scalar(
                out=K2[mt][:, c0:c0 + QB], in0=psk,
                scalar1=bk2_sb[:, mt:mt + 1], scalar2=None,
                op0=mybir.AluOpType.add)
            nc.vector.tensor_scalar(
                out=Q2[mt][:, c0:c0 + QB], in0=psq,
                scalar1=bq2_sb[:, mt:mt + 1], scalar2=None,
                op0=mybir.AluOpType.add)

        def v_dma(blk):
            c0 = blk * QB
            eng = nc.gpsimd if blk <= 1 else nc.sync
            xv_t = []
            for k in range(NDT):
                xv = xpool.tile([128, QB], bf16, tag="xt", name=f"xv{blk}_{k}")
                eng.dma_start(out=xv, in_=xvT[k * 128:(k + 1) * 128, c0:c0 + QB])
                if blk == 0:
                    eng.dma_start(out=wv_t[k], in_=wvT[k * 128:(k + 1) * 128, :])
                xv_t.append(xv)
            return xv_t

        def v_gen(blk, xv_t):
            """V projection for tokens [blk*QB, ...) -> V_sb[4blk..4blk+3]."""
            for w in range(2):
                psv = psP.tile([128, 2 * QB], fp32, tag="p2", name="psv")
                ps = [psv[:, 0:DPC], psv[:, QB:QB + DPC]]
                for k in range(NDT):
                    for i in range(2):
                        m2 = 2 * w + i
                        nc.tensor.matmul(out=ps[i], rhs=wv_t[k],
                                         lhsT=xv_t[k][:, m2 * 128:(m2 + 1) * 128],
                                         start=(k == 0), stop=False)
                        yield
                for i in range(2):
                    mt = SUB * blk + 2 * w + i
                    nc.tensor.matmul(out=ps[i], rhs=wvb_sb[0:1, :],
                                     lhsT=ones1[0:1, :], start=False, stop=True)
                    nc.vector.tensor_copy(
                        out=V_sb[mt][:, :, 0:64],
                        in_=ps[i].rearrange("p (h e) -> p h e", h=HPC))
                    nc.vector.memset(V_sb[mt][:, :, 64:65], 1.0)
                    yield

        def sc_head(qb, h, pts, pump):
            """Scores + exp + causal mask for one head of q block qb."""
            q0 = qb * QB
            mt, hh = divmod(h, 2)
            krows = slice(64 * hh, 64 * hh + 64)
            nfull = SUB * qb
            for i in range(nfull // 2):
                # two full k tiles share a 2-bank PSUM tile and one wide exp
                k0 = 2 * i
                s2 = psP.tile([128, 2 * QB], fp32, tag="p2", name=f"s2{h}_{i}")
                pt2 = ppool.tile([128, 2 * QB], bf16, tag="pt2",
                                 name=f"pt2{h}_{i}")
                for d in range(2):
                    nc.tensor.matmul(
                        out=s2[:, d * QB:(d + 1) * QB],
                        lhsT=K2[mt][krows, (k0 + d) * KT:(k0 + d + 1) * KT],
                        rhs=Q2[mt][krows, q0:q0 + QB],
                        start=True, stop=True)
                    pump(1)
                nc.scalar.activation(
                    out=pt2, in_=s2,
                    func=mybir.ActivationFunctionType.Exp)
                if use_kpm:
                    for d in range(2):
                        nc.vector.tensor_scalar(
                            out=pt2[:, d * QB:(d + 1) * QB],
                            in0=pt2[:, d * QB:(d + 1) * QB],
                            scalar1=kpm_sb[:, k0 + d:k0 + d + 1], scalar2=None,
                            op0=mybir.AluOpType.mult)
                pts[h, k0] = pt2[:, 0:QB]
                pts[h, k0 + 1] = pt2[:, QB:2 * QB]
                pump(4)
            for kti in range(nfull, nfull + SUB):
                o = 128 * (kti - nfull)
                s_ps = psM.tile([128, QB], fp32, tag="m", name=f"s{h}_{kti}")
                nc.tensor.matmul(
                    out=s_ps[:, o:QB],
                    lhsT=K2[mt][krows, kti * KT:(kti + 1) * KT],
                    rhs=Q2[mt][krows, q0 + o:q0 + QB],
                    start=True, stop=True)
                pt = ppool.tile([128, QB], bf16, tag="pt", name=f"pt{h}_{kti}")
                nc.scalar.activation(
                    out=pt[:, o:QB], in_=s_ps[:, o:QB],
                    func=mybir.ActivationFunctionType.Exp)
                nc.vector.tensor_mul(
                    out=pt[:, o:o + KT], in0=pt[:, o:o + KT], in1=tri_sb)
                if use_kpm:
                    nc.vector.tensor_scalar(
                        out=pt[:, o:QB], in0=pt[:, o:QB],
                        scalar1=kpm_sb[:, kti:kti + 1], scalar2=None,
                        op0=mybir.AluOpType.mult)
                pts[h, kti] = pt
                pump(3)

        def pv_gen(qb, p, pts):
            """PV for head pair p; ctx lands q-major; one transpose/chunk."""
            q0 = qb * QB
            h0, h1 = 2 * p, 2 * p + 1

            def flush(qs, cq):
                tr_ps = psC.tile([128, 128], bf16, tag="c")
                nc.tensor.transpose(out=tr_ps, in_=cq, identity=ident_sb)
                nc.vector.tensor_copy(
                    out=ctxT[p][:, q0 + qs * 128:q0 + (qs + 1) * 128],
                    in_=tr_ps)

            pend = None
            for qs in range(SUB):
                cq = cqpool.tile([128, 128], bf16, tag="cq")
                for j, h in enumerate((h0, h1)):
                    ctx_ps = psC.tile([128, 65], fp32, tag="c", name=f"ctx{j}")
                    for kti in range(SUB * qb + qs + 1):
                        nc.tensor.matmul(
                            out=ctx_ps,
                            lhsT=pts[h, kti][:, qs * 128:(qs + 1) * 128],
                            rhs=V_sb[kti][:, h, :],
                            start=(kti == 0), stop=(kti == SUB * qb + qs))
                    rcp = spool.tile([128, 1], fp32, tag="rcp")
                    nc.vector.reciprocal(out=rcp, in_=ctx_ps[:, 64:65])
                    nc.vector.tensor_scalar(
                        out=cq[:, 64 * j:64 * j + 64], in0=ctx_ps[:, 0:64],
                        scalar1=rcp, scalar2=None, op0=mybir.AluOpType.mult)
                    yield
                if pend is not None:
                    flush(*pend)
                pend = (qs, cq)
            flush(*pend)

        def op_gen(qb):
            """Out-proj rows [qb*QB, ...) -> partial -> this chunk's RS."""
            for mt in range(SUB * qb, SUB * qb + SUB):
                po = opool.tile([128, D], bf16, tag="po")
                for oc in range(D // QB):
                    ps = psM.tile([128, QB], fp32, tag="m", name="pso")
                    for t in range(NMT):
                        nc.tensor.matmul(
                            out=ps,
                            lhsT=ctxT[t][:, mt * 128:(mt + 1) * 128],
                            rhs=wo_sb[t][:, oc * QB:(oc + 1) * QB],
                            start=(t == 0), stop=(t == NMT - 1))
                        yield
                    nc.vector.tensor_add(
                        out=po[:, oc * QB:(oc + 1) * QB],
                        in0=ps, in1=bo2_sb[:, oc * QB:(oc + 1) * QB])
                nc.sync.dma_start(
                    out=partial_dram[mt * 128:(mt + 1) * 128, :], in_=po)
            # chunks qb0/qb1 ReduceScatter alone (early, hidden under
            # compute); qb2+qb3 merge into one collective fired after qb3 —
            # the two tail chunks land nearly together, so one 28us RS beats
            # two serialized 21.5us ones.
            if qb <= 1:
                nc.gpsimd.collective_compute(
                    "ReduceScatter",
                    mybir.AluOpType.add,
                    replica_groups=groups,
                    ins=[partial_dram[qb * QB:(qb + 1) * QB, :]],
                    outs=[rs_out[qb * 128:(qb + 1) * 128, :]],
                )
            elif qb == 3:
                nc.gpsimd.collective_compute(
                    "ReduceScatter",
                    mybir.AluOpType.add,
                    replica_groups=groups,
                    ins=[partial_dram[2 * QB:4 * QB, :]],
                    outs=[rs_out[256:512, :]],
                )

        # Weaved emission: score tiles are exp-paced (the scalar engine
        # drains one [128,512] exp per ~610ns vs the 213ns score matmul), so
        # after each score tile we pump ~2 steps of queued exp-free work
        # (projections, PV, out-proj+RS) into the PE stream.  Queue is FIFO
        # in schedule order, so data dependencies (V before PV, PV before
        # out-proj) drain in order; kq blocks a score head reads are force-
        # drained first so readers never precede their writers.
        sched = [
            ("kq", 0),
            ("sc", 0, 0), ("sc", 0, 1),
            ("v", 0), ("kq", 1),
            ("sc", 0, 2), ("sc", 0, 3),
            ("kq", 2), ("v", 1),
            ("sc", 1, 0), ("sc", 1, 1), ("pv", 0, 0),
            ("sc", 1, 2), ("sc", 1, 3), ("pv", 0, 1),
            ("op", 0),
            ("kq", 3), ("v", 2),
            ("sc", 2, 0), ("sc", 2, 1), ("pv", 1, 0),
            ("sc", 2, 2), ("sc", 2, 3), ("pv", 1, 1),
            ("op", 1),
            ("v", 3),
            ("sc", 3, 0), ("sc", 3, 1), ("pv", 2, 0),
            ("sc", 3, 2), ("pv", 2, 1),
            ("op", 2),
            ("sc", 3, 3),
            ("pv", 3, 0), ("pv", 3, 1),
            ("op", 3),
        ]
        pts_all = {qb: {} for qb in range(NQB)}
        queue = deque()

        def pump(n):
            while n > 0 and queue:
                try:
                    next(queue[0][1])
                    n -= 1
                except StopIteration:
                    queue.popleft()

        def drain_through(pred):
            """Fully drain queue head through the last unit matching pred."""
            if not any(pred(key) for key, _ in queue):
                return
            last = max(i for i, (key, _) in enumerate(queue) if pred(key))
            for _ in range(last + 1):
                key, gen = queue.popleft()
                for _ in gen:
                    pass

        for unit in sched:
            kind = unit[0]
            if kind == "kq":
                xk_t, xq_t = kq_dma(unit[1])
                for mt in range(NMT):
                    queue.append((("kq", unit[1], mt),
                                  kq_gen(unit[1], mt, xk_t, xq_t)))
            elif kind == "v":
                xv_t = v_dma(unit[1])
                queue.append((unit, v_gen(unit[1], xv_t)))
            elif kind == "pv":
                queue.append((unit, pv_gen(unit[1], unit[2], pts_all[unit[1]])))
            elif kind == "op":
                queue.append((unit, op_gen(unit[1])))
            else:
                qb, h = unit[1], unit[2]
                drain_through(lambda key: key[0] == "kq" and key[1] <= qb
                              and key[2] <= h // 2)
                sc_head(qb, h, pts_all[qb], pump)
        while queue:
            pump(1000)

        # rs_out -> out via SBUF bounces.  Chunks 0/1 ride the idle sync
        # queue and hide under later compute; the tail-exposed merged chunk
        # is split across the sync and scalar queues so its two row tiles
        # bounce in parallel.
        for qb in range(2):
            osb = opool.tile([128, D], bf16, tag="osb")
            nc.sync.dma_start(out=osb, in_=rs_out[qb * 128:(qb + 1) * 128, :])
            nc.sync.dma_start(out=out_ext[qb * 128:(qb + 1) * 128, :], in_=osb)
        for i, eng in enumerate((nc.sync, nc.scalar)):
            r0 = 256 + 128 * i
            osb = opool.tile([128, D], bf16, tag="osb")
            eng.dma_start(out=osb, in_=rs_out[r0:r0 + 128, :])
            eng.dma_start(out=out_ext[r0:r0 + 128, :], in_=osb)

    nc.compile()
    return nc


def prep_core_inputs(inputs, core):
    """Host-side shard/layout prep for one core.  Pure layout + dtype work."""
    b, g = divmod(core, G)
    sl = slice(g * DPC, (g + 1) * DPC)
    s = 1.0 / np.sqrt(DH)

    def xT(x):
        return np.ascontiguousarray(np.asarray(x)[b].T).astype(BF16)

    def wT(w, scale=1.0):
        wt = np.asarray(w)[sl, :].T.astype(np.float32)
        return (wt * scale).astype(BF16)

    def b2(bias, scale=1.0):
        bb = (np.asarray(bias)[sl].astype(np.float32) * scale)
        return np.ascontiguousarray(bb.reshape(NMT, 128).T)

    kpm = np.asarray(inputs["key_padding_mask"])
    tri = (np.arange(KT)[:, None] <= np.arange(KT)[None, :]).astype(BF16)

    d = {
        "xqT": xT(inputs["q_input"]),
        "xkT": xT(inputs["k_input"]),
        "xvT": xT(inputs["v_input"]),
        "wqT": wT(inputs["wq"], s),
        "wkT": wT(inputs["wk"]),
        "wvT": wT(inputs["wv"]),
        "wvb": np.asarray(inputs["bv"])[sl].astype(BF16)[None, :],
        "woT": np.ascontiguousarray(np.asarray(inputs["wo"]).T[sl, :]).astype(BF16),
        "bq2": b2(inputs["bq"], s),
        "bk2": b2(inputs["bk"]),
        "tri": tri,
        "ident": np.eye(128, dtype=BF16),
        "bo2": np.tile(np.asarray(inputs["bo"]).astype(np.float32) / G, (128, 1)),
    }
    if kpm.any():
        # kpmT[p, kti] = 0.0 for padded key (128*kti + p), else 1.0
        d["kpmT"] = np.ascontiguousarray(
            1.0 - kpm[b].astype(np.float32).reshape(SEQ // 128, 128).T)
    return d


def assemble_output(core_outs):
    """core 4b+j: chunks qb0/qb1 hold reduced rows [512qb+128j, +128);
    the merged qb2+qb3 chunk holds rows [1024+256j, +256)."""
    out = np.empty((B, SEQ, D), dtype=np.float32)
    for core in range(NCORES):
        b, j = divmod(core, G)
        co = np.asarray(core_outs[core]).astype(np.float32)
        for qb in range(2):
            r0 = QB * qb + 128 * j
            out[b, r0:r0 + 128, :] = co[qb * 128:(qb + 1) * 128]
        r0 = 2 * QB + 256 * j
        out[b, r0:r0 + 256, :] = co[256:512]
    return out


_CACHED = {}


def _get_nc(use_kpm=False):
    if use_kpm not in _CACHED:
        _CACHED[use_kpm] = build_program(use_kpm=use_kpm)
    return _CACHED[use_kpm]


def kernel(**inputs) -> np.ndarray:
    use_kpm = bool(np.asarray(inputs["key_padding_mask"]).any())
    nc = _get_nc(use_kpm)
    in_maps = [prep_core_inputs(inputs, core) for core in range(NCORES)]
    res = run_bass_kernel_spmd(nc, in_maps, core_ids=list(range(NCORES)))
    return assemble_output([res.results[c]["out"] for c in range(NCORES)])


if __name__ == "__main__":
    nc = build_program()
    print("program built ok")



# revision 70
# speedup vs baseline: 1.0076x; 1.0076x over previous
"""Trainium2 Bass kernel for nn_AttentionUnit (multi-head attention block), v2.

Reference math (B=2, S=2048, D=1024, H=16 heads, d_head=64, fp32):
    Q = q @ wq.T + bq ; K = k @ wk.T + bk ; V = v @ wv.T + bv
    S = QK^T / 8  (per head), causal mask + key-padding mask
    out = softmax(S) @ V  -> concat heads -> @ wo.T + bo

Sharding (8 cores): data-parallel over batch (2 groups of 4 cores),
tensor-parallel over heads (4 heads/core).  Column-parallel QKV,
row-parallel wo; partial outputs are summed with in-group
ReduceScatters (per q block for qb0/qb1, one merged collective for
qb2+qb3) that pipeline behind compute; the host reassembles the
row-range outputs.

Schedule: all emission units (projection blocks, score heads, PV pairs,
out-proj) are generators woven into one PE stream.  Score tiles are
exp-paced (one [128,512] Exp drains in ~610ns vs the 213ns score
matmul), so after each score tile ~3 steps of queued exp-free matmul
work are pumped into the stream; the scalar engine then runs the exp
stream back-to-back while the PE stays busy, and out-proj chunks finish
early enough that only the last collective is tail-exposed.

Other layout choices:
  - K/Q projections run contraction-outer over 4 PSUM banks, fed by
    consumption-ordered DMAs, so the PE starts ~1us into the kernel and
    the softmax Exp stream (the scalar-engine bottleneck) starts ~8us.
  - Q/K biases are applied during PSUM eviction (per-partition
    tensor_scalar add) instead of riding the matmul contraction; K2/Q2
    activations are kept mt-major [128, S] so eviction is one op/bank.
  - Scores are computed transposed (S^T[k, q]) per head; exp on the
    scalar engine; causal masking multiplies only the 128-wide diagonal
    sub-tile by a lower-triangle constant.  P^T chunks are the
    stationary operand of PV, so ctx lands q-major and the softmax
    division is a per-partition reciprocal+scale; head pairs share one
    [128,128] PE transpose into ctx^T for the output projection.
  - Engine placement: scalar = exp + small consts + final output
    bounce; DVE = all PSUM evictions, masks, ctx normalize chain;
    gpsimd = collectives + early-phase input DMAs; sync = bulk input
    DMAs + partial-output writes.
"""

import os
import sys
from collections import deque
from contextlib import ExitStack

import numpy as np

try:
    import concourse.bass as bass
except ImportError:  # harness containers keep the repo at /opt/trn_rl_repo
    for _p in ("/opt/trn_rl_repo", "/root/.axon_site/_ro/trn_rl_repo"):
        if os.path.isdir(_p) and _p not in sys.path:
            sys.path.insert(0, _p)
    import concourse.bass as bass

from concourse import bacc

import ml_dtypes
import concourse.mybir as mybir
import concourse.tile as tile
from concourse.bass_utils import run_bass_kernel_spmd

BF16 = ml_dtypes.bfloat16

B = 2
SEQ = 2048
D = 1024
H = 16
DH = 64
NCORES = 8
G = 4            # tensor-parallel group size (cores per batch)
HPC = H // G     # heads per core
DPC = HPC * DH   # head dims per core (256)
QB = 512         # q block width
KT = 128         # k tile height
NMT = DPC // 128  # mt tiles of per-core head dims (2)
NDT = D // 128    # contraction tiles of the model dim (8)
NQB = SEQ // QB   # q blocks (4)
SUB = QB // KT    # k tiles per q block on the diagonal (4)


def build_program(use_kpm=False):
    """Emit the SPMD program (identical on all 8 cores)."""
    fp32 = mybir.dt.float32
    bf16 = mybir.dt.bfloat16

    nc = bacc.Bacc(num_devices=NCORES)

    xqT = nc.declare_dram_parameter("xqT", [D, SEQ], bf16, False)
    xkT = nc.declare_dram_parameter("xkT", [D, SEQ], bf16, False)
    xvT = nc.declare_dram_parameter("xvT", [D, SEQ], bf16, False)
    wqT = nc.declare_dram_parameter("wqT", [D, DPC], bf16, False)
    wkT = nc.declare_dram_parameter("wkT", [D, DPC], bf16, False)
    wvT = nc.declare_dram_parameter("wvT", [D, DPC], bf16, False)
    wvb = nc.declare_dram_parameter("wvb", [1, DPC], bf16, False)
    woT = nc.declare_dram_parameter("woT", [DPC, D], bf16, False)
    bq2_d = nc.declare_dram_parameter("bq2", [128, NMT], fp32, False)
    bk2_d = nc.declare_dram_parameter("bk2", [128, NMT], fp32, False)
    tri_d = nc.declare_dram_parameter("tri", [KT, KT], bf16, False)
    ident_d = nc.declare_dram_parameter("ident", [128, 128], bf16, False)
    bo2_d = nc.declare_dram_parameter("bo2", [128, D], fp32, False)
    kpm_d = (nc.declare_dram_parameter("kpmT", [128, SEQ // 128], fp32, False)
             if use_kpm else None)
    out_ext = nc.declare_dram_parameter("out", [SEQ // G, D], bf16, isOutput=True)

    partial_dram = nc.dram_tensor("partial", [SEQ, D], bf16)
    rs_out = nc.dram_tensor("rs_out", [SEQ // G, D], bf16)

    groups = [[0, 1, 2, 3], [4, 5, 6, 7]]

    with ExitStack() as ctx:
        tc = ctx.enter_context(tile.TileContext(nc, num_cores=NCORES))

        xpool = ctx.enter_context(tc.tile_pool(name="xp", bufs=28))
        persist = ctx.enter_context(tc.tile_pool(name="persist", bufs=1))
        ppool = ctx.enter_context(tc.tile_pool(name="pp", bufs=34))
        cqpool = ctx.enter_context(tc.tile_pool(name="cq", bufs=4))
        opool = ctx.enter_context(tc.tile_pool(name="op", bufs=4))
        spool = ctx.enter_context(tc.tile_pool(name="sp", bufs=8))
        psP = ctx.enter_context(tc.tile_pool(name="psP", bufs=2, space="PSUM"))
        psM = ctx.enter_context(tc.tile_pool(name="psM", bufs=2, space="PSUM"))
        psC = ctx.enter_context(tc.tile_pool(name="psC", bufs=2, space="PSUM"))

        # ---- small constants (spread over queues; all early) ----
        tri_sb = persist.tile([KT, KT], bf16, tag="tri")
        nc.scalar.dma_start(out=tri_sb, in_=tri_d[:, :])
        ident_sb = persist.tile([128, 128], bf16, tag="ident")
        nc.scalar.dma_start(out=ident_sb, in_=ident_d[:, :])
        bq2_sb = persist.tile([128, NMT], fp32, tag="bq2")
        nc.scalar.dma_start(out=bq2_sb, in_=bq2_d[:, :])
        bk2_sb = persist.tile([128, NMT], fp32, tag="bk2")
        nc.scalar.dma_start(out=bk2_sb, in_=bk2_d[:, :])
        bo2_sb = persist.tile([128, D], fp32, tag="bo2")
        nc.scalar.dma_start(out=bo2_sb, in_=bo2_d[:, :])
        wvb_sb = persist.tile([1, DPC], bf16, tag="wvb")
        nc.scalar.dma_start(out=wvb_sb, in_=wvb[0:1, :])
        ones1 = persist.tile([1, 128], bf16, tag="ones1")
        nc.vector.memset(ones1, 1.0)
        kpm_sb = None
        if use_kpm:
            kpm_sb = persist.tile([128, SEQ // 128], fp32, tag="kpm")
            nc.scalar.dma_start(out=kpm_sb, in_=kpm_d[:, :])
        wo_sb = [persist.tile([128, D], bf16, tag=f"wo{t}", name=f"wo{t}")
                 for t in range(NMT)]
        for t in range(NMT):
            nc.scalar.dma_start(out=wo_sb[t], in_=woT[t * 128:(t + 1) * 128, :])

        # ---- persistent weights (consumed every block; load once) ----
        wk_t = [persist.tile([128, DPC], bf16, tag=f"wk{k}", name=f"wk{k}")
                for k in range(NDT)]
        wq_t = [persist.tile([128, DPC], bf16, tag=f"wq{k}", name=f"wq{k}")
                for k in range(NDT)]
        wv_t = [persist.tile([128, DPC], bf16, tag=f"wv{k}", name=f"wv{k}")
                for k in range(NDT)]

        # ---- persistent activations ----
        # K2/Q2 are mt-major: rows = the 128 head dims of heads (2mt, 2mt+1).
        K2 = [persist.tile([128, SEQ], bf16, tag=f"K2{t}", name=f"K2{t}")
              for t in range(NMT)]
        Q2 = [persist.tile([128, SEQ], bf16, tag=f"Q2{t}", name=f"Q2{t}")
              for t in range(NMT)]
        V_sb = [persist.tile([128, HPC, 65], bf16, tag=f"V{m}", name=f"V{m}")
                for m in range(SEQ // KT)]
        ctxT = [persist.tile([128, SEQ], bf16, tag=f"ctxT{t}", name=f"ctxT{t}")
                for t in range(NMT)]

        def kq_dma(blk):
            """Issue K/Q input DMAs for q/k columns [blk*QB, (blk+1)*QB)."""
            c0 = blk * QB
            eng = nc.gpsimd if blk <= 1 else nc.sync
            xk_t, xq_t = [], []
            for k in range(NDT):
                xk = xpool.tile([128, QB], bf16, tag="xt", name=f"xk{blk}_{k}")
                nc.sync.dma_start(out=xk, in_=xkT[k * 128:(k + 1) * 128, c0:c0 + QB])
                if blk == 0:
                    nc.sync.dma_start(out=wk_t[k], in_=wkT[k * 128:(k + 1) * 128, :])
                xq = xpool.tile([128, QB], bf16, tag="xt", name=f"xq{blk}_{k}")
                eng.dma_start(out=xq, in_=xqT[k * 128:(k + 1) * 128, c0:c0 + QB])
                if blk == 0:
                    eng.dma_start(out=wq_t[k], in_=wqT[k * 128:(k + 1) * 128, :])
                xk_t.append(xk)
                xq_t.append(xq)
            return xk_t, xq_t

        def kq_gen(blk, mt, xk_t, xq_t):
            """K and Q projection matmuls for one mt half (yield/matmul).
            Splitting per-mt lets scores for heads 2mt/2mt+1 start after
            only this half's eviction."""
            c0 = blk * QB
            pskq = psP.tile([128, 2 * QB], fp32, tag="p2", name="pskq")
            psk, psq = pskq[:, 0:QB], pskq[:, QB:2 * QB]
            for k in range(NDT):
                st, sp = (k == 0), (k == NDT - 1)
                nc.tensor.matmul(out=psk, rhs=xk_t[k],
                                 lhsT=wk_t[k][:, mt * 128:(mt + 1) * 128],
                                 start=st, stop=sp)
                yield
                nc.tensor.matmul(out=psq, rhs=xq_t[k],
                                 lhsT=wq_t[k][:, mt * 128:(mt + 1) * 128],
                                 start=st, stop=sp)
                yield
            nc.vector.tensor_scalar(
                out=K2[mt][:, c0:c0 + QB], in0=psk,
                scalar1=bk2_sb[:, mt:mt + 1], scalar2=None,
                op0=mybir.AluOpType.add)
            nc.vector.tensor_scalar(
                out=Q2[mt][:, c0:c0 + QB], in0=psq,
                scalar1=bq2_sb[:, mt:mt + 1], scalar2=None,
                op0=mybir.AluOpType.add)

        def v_dma(blk):
            c0 = blk * QB
            eng = nc.gpsimd if blk <= 1 else nc.sync
            xv_t = []
            for k in range(NDT):
                xv = xpool.tile([128, QB], bf16, tag="xt", name=f"xv{blk}_{k}")
                eng.dma_start(out=xv, in_=xvT[k * 128:(k + 1) * 128, c0:c0 + QB])
                if blk == 0:
                    eng.dma_start(out=wv_t[k], in_=wvT[k * 128:(k + 1) * 128, :])
                xv_t.append(xv)
            return xv_t

        def v_gen(blk, xv_t):
            """V projection for tokens [blk*QB, ...) -> V_sb[4blk..4blk+3]."""
            for w in range(2):
                psv = psP.tile([128, 2 * QB], fp32, tag="p2", name="psv")
                ps = [psv[:, 0:DPC], psv[:, QB:QB + DPC]]
                for k in range(NDT):
                    for i in range(2):
                        m2 = 2 * w + i
                        nc.tensor.matmul(out=ps[i], rhs=wv_t[k],
                                         lhsT=xv_t[k][:, m2 * 128:(m2 + 1) * 128],
                                         start=(k == 0), stop=False)
                        yield
                for i in range(2):
                    mt = SUB * blk + 2 * w + i
                    nc.tensor.matmul(out=ps[i], rhs=wvb_sb[0:1, :],
                                     lhsT=ones1[0:1, :], start=False, stop=True)
                    nc.vector.tensor_copy(
                        out=V_sb[mt][:, :, 0:64],
                        in_=ps[i].rearrange("p (h e) -> p h e", h=HPC))
                    nc.vector.memset(V_sb[mt][:, :, 64:65], 1.0)
                    yield

        def sc_head(qb, h, pts, pump):
            """Scores + exp + causal mask for one head of q block qb."""
            q0 = qb * QB
            mt, hh = divmod(h, 2)
            krows = slice(64 * hh, 64 * hh + 64)
            nfull = SUB * qb
            for i in range(nfull // 2):
                # two full k tiles share a 2-bank PSUM tile and one wide exp
                k0 = 2 * i
                s2 = psP.tile([128, 2 * QB], fp32, tag="p2", name=f"s2{h}_{i}")
                pt2 = ppool.tile([128, 2 * QB], bf16, tag="pt2",
                                 name=f"pt2{h}_{i}")
                for d in range(2):
                    nc.tensor.matmul(
                        out=s2[:, d * QB:(d + 1) * QB],
                        lhsT=K2[mt][krows, (k0 + d) * KT:(k0 + d + 1) * KT],
                        rhs=Q2[mt][krows, q0:q0 + QB],
                        start=True, stop=True)
                    pump(1)
                nc.scalar.activation(
                    out=pt2, in_=s2,
                    func=mybir.ActivationFunctionType.Exp)
                if use_kpm:
                    for d in range(2):
                        nc.vector.tensor_scalar(
                            out=pt2[:, d * QB:(d + 1) * QB],
                            in0=pt2[:, d * QB:(d + 1) * QB],
                            scalar1=kpm_sb[:, k0 + d:k0 + d + 1], scalar2=None,
                            op0=mybir.AluOpType.mult)
                pts[h, k0] = pt2[:, 0:QB]
                pts[h, k0 + 1] = pt2[:, QB:2 * QB]
                pump(3)
            for kti in range(nfull, nfull + SUB):
                o = 128 * (kti - nfull)
                s_ps = psM.tile([128, QB], fp32, tag="m", name=f"s{h}_{kti}")
                nc.tensor.matmul(
                    out=s_ps[:, o:QB],
                    lhsT=K2[mt][krows, kti * KT:(kti + 1) * KT],
                    rhs=Q2[mt][krows, q0 + o:q0 + QB],
                    start=True, stop=True)
                pt = ppool.tile([128, QB], bf16, tag="pt", name=f"pt{h}_{kti}")
                nc.scalar.activation(
                    out=pt[:, o:QB], in_=s_ps[:, o:QB],
                    func=mybir.ActivationFunctionType.Exp)
                nc.vector.tensor_mul(
                    out=pt[:, o:o + KT], in0=pt[:, o:o + KT], in1=tri_sb)
                if use_kpm:
                    nc.vector.tensor_scalar(
                        out=pt[:, o:QB], in0=pt[:, o:QB],
                        scalar1=kpm_sb[:, kti:kti + 1], scalar2=None,
                        op0=mybir.AluOpType.mult)
                pts[h, kti] = pt
                pump(2)

        def pv_gen(qb, p, pts):
            """PV for head pair p; ctx lands q-major; one transpose/chunk."""
            q0 = qb * QB
            h0, h1 = 2 * p, 2 * p + 1

            def flush(qs, cq):
                tr_ps = psC.tile([128, 128], bf16, tag="c")
                nc.tensor.transpose(out=tr_ps, in_=cq, identity=ident_sb)
                nc.vector.tensor_copy(
                    out=ctxT[p][:, q0 + qs * 128:q0 + (qs + 1) * 128],
                    in_=tr_ps)

            pend = None
            for qs in range(SUB):
                cq = cqpool.tile([128, 128], bf16, tag="cq")
                for j, h in enumerate((h0, h1)):
                    ctx_ps = psC.tile([128, 65], fp32, tag="c", name=f"ctx{j}")
                    for kti in range(SUB * qb + qs + 1):
                        nc.tensor.matmul(
                            out=ctx_ps,
                            lhsT=pts[h, kti][:, qs * 128:(qs + 1) * 128],
                            rhs=V_sb[kti][:, h, :],
                            start=(kti == 0), stop=(kti == SUB * qb + qs))
                    rcp = spool.tile([128, 1], fp32, tag="rcp")
                    nc.vector.reciprocal(out=rcp, in_=ctx_ps[:, 64:65])
                    nc.vector.tensor_scalar(
                        out=cq[:, 64 * j:64 * j + 64], in0=ctx_ps[:, 0:64],
                        scalar1=rcp, scalar2=None, op0=mybir.AluOpType.mult)
                    yield
                if pend is not None:
                    flush(*pend)
                pend = (qs, cq)
            flush(*pend)

        def op_gen(qb):
            """Out-proj rows [qb*QB, ...) -> partial -> this chunk's RS."""
            for mt in range(SUB * qb, SUB * qb + SUB):
                po = opool.tile([128, D], bf16, tag="po")
                for oc in range(D // QB):
                    ps = psM.tile([128, QB], fp32, tag="m", name="pso")
                    for t in range(NMT):
                        nc.tensor.matmul(
                            out=ps,
                            lhsT=ctxT[t][:, mt * 128:(mt + 1) * 128],
                            rhs=wo_sb[t][:, oc * QB:(oc + 1) * QB],
                            start=(t == 0), stop=(t == NMT - 1))
                        yield
                    nc.vector.tensor_add(
                        out=po[:, oc * QB:(oc + 1) * QB],
                        in0=ps, in1=bo2_sb[:, oc * QB:(oc + 1) * QB])
                nc.sync.dma_start(
                    out=partial_dram[mt * 128:(mt + 1) * 128, :], in_=po)
            # chunks qb0/qb1 ReduceScatter alone (early, hidden under
            # compute); qb2+qb3 merge into one collective fired after qb3 —
            # the two tail chunks land nearly together, so one 28us RS beats
            # two serialized 21.5us ones.
            if qb <= 1:
                nc.gpsimd.collective_compute(
                    "ReduceScatter",
                    mybir.AluOpType.add,
                    replica_groups=groups,
                    ins=[partial_dram[qb * QB:(qb + 1) * QB, :]],
                    outs=[rs_out[qb * 128:(qb + 1) * 128, :]],
                )
            elif qb == 3:
                nc.gpsimd.collective_compute(
                    "ReduceScatter",
                    mybir.AluOpType.add,
                    replica_groups=groups,
                    ins=[partial_dram[2 * QB:4 * QB, :]],
                    outs=[rs_out[256:512, :]],
                )

        # Weaved emission: score tiles are exp-paced (the scalar engine
        # drains one [128,512] exp per ~610ns vs the 213ns score matmul), so
        # after each score tile we pump ~2 steps of queued exp-free work
        # (projections, PV, out-proj+RS) into the PE stream.  Queue is FIFO
        # in schedule order, so data dependencies (V before PV, PV before
        # out-proj) drain in order; kq blocks a score head reads are force-
        # drained first so readers never precede their writers.
        sched = [
            ("kq", 0),
            ("sc", 0, 0), ("sc", 0, 1),
            ("v", 0), ("kq", 1),
            ("sc", 0, 2), ("sc", 0, 3),
            ("kq", 2), ("v", 1),
            ("sc", 1, 0), ("sc", 1, 1), ("pv", 0, 0),
            ("sc", 1, 2), ("sc", 1, 3), ("pv", 0, 1),
            ("op", 0),
            ("kq", 3), ("v", 2),
            ("sc", 2, 0), ("sc", 2, 1), ("pv", 1, 0),
            ("sc", 2, 2), ("sc", 2, 3), ("pv", 1, 1),
            ("op", 1),
            ("v", 3),
            ("sc", 3, 0), ("sc", 3, 1), ("pv", 2, 0),
            ("sc", 3, 2), ("pv", 2, 1),
            ("op", 2),
            ("sc", 3, 3),
            ("pv", 3, 0), ("pv", 3, 1),
            ("op", 3),
        ]
        pts_all = {qb: {} for qb in range(NQB)}
        queue = deque()

        def pump(n):
            while n > 0 and queue:
                try:
                    next(queue[0][1])
                    n -= 1
                except StopIteration:
                    queue.popleft()

        def drain_through(pred):
            """Fully drain queue head through the last unit matching pred."""
            if not any(pred(key) for key, _ in queue):
                return
            last = max(i for i, (key, _) in enumerate(queue) if pred(key))
            for _ in range(last + 1):
                key, gen = queue.popleft()
                for _ in gen:
                    pass

        for unit in sched:
            kind = unit[0]
            if kind == "kq":
                xk_t, xq_t = kq_dma(unit[1])
                for mt in range(NMT):
                    queue.append((("kq", unit[1], mt),
                                  kq_gen(unit[1], mt, xk_t, xq_t)))
            elif kind == "v":
                xv_t = v_dma(unit[1])
                queue.append((unit, v_gen(unit[1], xv_t)))
            elif kind == "pv":
                queue.append((unit, pv_gen(unit[1], unit[2], pts_all[unit[1]])))
            elif kind == "op":
                queue.append((unit, op_gen(unit[1])))
            else:
                qb, h = unit[1], unit[2]
                drain_through(lambda key: key[0] == "kq" and key[1] <= qb
                              and key[2] <= h // 2)
                sc_head(qb, h, pts_all[qb], pump)
        while queue:
            pump(1000)

        # rs_out -> out via SBUF bounces.  Chunks 0/1 ride the idle sync
        # queue and hide under later compute; the tail-exposed merged chunk
        # is split across the sync and scalar queues so its two row tiles
        # bounce in parallel.
        for qb in range(2):
            osb = opool.tile([128, D], bf16, tag="osb")
            nc.sync.dma_start(out=osb, in_=rs_out[qb * 128:(qb + 1) * 128, :])
            nc.sync.dma_start(out=out_ext[qb * 128:(qb + 1) * 128, :], in_=osb)
        for i, eng in enumerate((nc.sync, nc.scalar)):
            r0 = 256 + 128 * i
            osb = opool.tile([128, D], bf16, tag="osb")
            eng.dma_start(out=osb, in_=rs_out[r0:r0 + 128, :])
            eng.dma_start(out=out_ext[r0:r0 + 128, :], in_=osb)

    nc.compile()
    return nc


def prep_core_inputs(inputs, core):
    """Host-side shard/layout prep for one core.  Pure layout + dtype work."""
    b, g = divmod(core, G)
    sl = slice(g * DPC, (g + 1) * DPC)
    s = 1.0 / np.sqrt(DH)

    def xT(x):
        return np.ascontiguousarray(np.asarray(x)[b].T).astype(BF16)

    def wT(w, scale=1.0):
        wt = np.asarray(w)[sl, :].T.astype(np.float32)
        return (wt * scale).astype(BF16)

    def b2(bias, scale=1.0):
        bb = (np.asarray(bias)[sl].astype(np.float32) * scale)
        return np.ascontiguousarray(bb.reshape(NMT, 128).T)

    kpm = np.asarray(inputs["key_padding_mask"])
    tri = (np.arange(KT)[:, None] <= np.arange(KT)[None, :]).astype(BF16)

    d = {
        "xqT": xT(inputs["q_input"]),
        "xkT": xT(inputs["k_input"]),
        "xvT": xT(inputs["v_input"]),
        "wqT": wT(inputs["wq"], s),
        "wkT": wT(inputs["wk"]),
        "wvT": wT(inputs["wv"]),
        "wvb": np.asarray(inputs["bv"])[sl].astype(BF16)[None, :],
        "woT": np.ascontiguousarray(np.asarray(inputs["wo"]).T[sl, :]).astype(BF16),
        "bq2": b2(inputs["bq"], s),
        "bk2": b2(inputs["bk"]),
        "tri": tri,
        "ident": np.eye(128, dtype=BF16),
        "bo2": np.tile(np.asarray(inputs["bo"]).astype(np.float32) / G, (128, 1)),
    }
    if kpm.any():
        # kpmT[p, kti] = 0.0 for padded key (128*kti + p), else 1.0
        d["kpmT"] = np.ascontiguousarray(
            1.0 - kpm[b].astype(np.float32).reshape(SEQ // 128, 128).T)
    return d


def assemble_output(core_outs):
    """core 4b+j: chunks qb0/qb1 hold reduced rows [512qb+128j, +128);
    the merged qb2+qb3 chunk holds rows [1024+256j, +256)."""
    out = np.empty((B, SEQ, D), dtype=np.float32)
    for core in range(NCORES):
        b, j = divmod(core, G)
        co = np.asarray(core_outs[core]).astype(np.float32)
        for qb in range(2):
            r0 = QB * qb + 128 * j
            out[b, r0:r0 + 128, :] = co[qb * 128:(qb + 1) * 128]
        r0 = 2 * QB + 256 * j
        out[b, r0:r0 + 256, :] = co[256:512]
    return out


_CACHED = {}


def _get_nc(use_kpm=False):
    if use_kpm not in _CACHED:
        _CACHED[use_kpm] = build_program(use_kpm=use_kpm)
    return _CACHED[use_kpm]


def kernel(**inputs) -> np.ndarray:
    use_kpm = bool(np.asarray(inputs["key_padding_mask"]).any())
    nc = _get_nc(use_kpm)
    in_maps = [prep_core_inputs(inputs, core) for core in range(NCORES)]
    res = run_bass_kernel_spmd(nc, in_maps, core_ids=list(range(NCORES)))
    return assemble_output([res.results[c]["out"] for c in range(NCORES)])


if __name__ == "__main__":
    nc = build_program()
    print("program built ok")



# revision 71
# speedup vs baseline: 1.0148x; 1.0072x over previous
"""Trainium2 Bass kernel for nn_AttentionUnit (multi-head attention block), v2.

Reference math (B=2, S=2048, D=1024, H=16 heads, d_head=64, fp32):
    Q = q @ wq.T + bq ; K = k @ wk.T + bk ; V = v @ wv.T + bv
    S = QK^T / 8  (per head), causal mask + key-padding mask
    out = softmax(S) @ V  -> concat heads -> @ wo.T + bo

Sharding (8 cores): data-parallel over batch (2 groups of 4 cores),
tensor-parallel over heads (4 heads/core).  Column-parallel QKV,
row-parallel wo; partial outputs are summed with in-group
ReduceScatters (per q block for qb0/qb1, one merged collective for
qb2+qb3) that pipeline behind compute; the host reassembles the
row-range outputs.

Schedule: all emission units (projection blocks, score heads, PV pairs,
out-proj) are generators woven into one PE stream.  Score tiles are
exp-paced (one [128,512] Exp drains in ~610ns vs the 213ns score
matmul), so after each score tile ~3 steps of queued exp-free matmul
work are pumped into the stream; the scalar engine then runs the exp
stream back-to-back while the PE stays busy, and out-proj chunks finish
early enough that only the last collective is tail-exposed.

Other layout choices:
  - K/Q projections run contraction-outer over 4 PSUM banks, fed by
    consumption-ordered DMAs, so the PE starts ~1us into the kernel and
    the softmax Exp stream (the scalar-engine bottleneck) starts ~8us.
  - Q/K biases are applied during PSUM eviction (per-partition
    tensor_scalar add) instead of riding the matmul contraction; K2/Q2
    activations are kept mt-major [128, S] so eviction is one op/bank.
  - Scores are computed transposed (S^T[k, q]) per head; exp on the
    scalar engine; causal masking multiplies only the 128-wide diagonal
    sub-tile by a lower-triangle constant.  P^T chunks are the
    stationary operand of PV, so ctx lands q-major and the softmax
    division is a per-partition reciprocal+scale; head pairs share one
    [128,128] PE transpose into ctx^T for the output projection.
  - Engine placement: scalar = exp + small consts + final output
    bounce; DVE = all PSUM evictions, masks, ctx normalize chain;
    gpsimd = collectives + early-phase input DMAs; sync = bulk input
    DMAs + partial-output writes.
"""

import os
import sys
from collections import deque
from contextlib import ExitStack

import numpy as np

try:
    import concourse.bass as bass
except ImportError:  # harness containers keep the repo at /opt/trn_rl_repo
    for _p in ("/opt/trn_rl_repo", "/root/.axon_site/_ro/trn_rl_repo"):
        if os.path.isdir(_p) and _p not in sys.path:
            sys.path.insert(0, _p)
    import concourse.bass as bass

from concourse import bacc

import ml_dtypes
import concourse.mybir as mybir
import concourse.tile as tile
from concourse.bass_utils import run_bass_kernel_spmd

BF16 = ml_dtypes.bfloat16

B = 2
SEQ = 2048
D = 1024
H = 16
DH = 64
NCORES = 8
G = 4            # tensor-parallel group size (cores per batch)
HPC = H // G     # heads per core
DPC = HPC * DH   # head dims per core (256)
QB = 512         # q block width
KT = 128         # k tile height
NMT = DPC // 128  # mt tiles of per-core head dims (2)
NDT = D // 128    # contraction tiles of the model dim (8)
NQB = SEQ // QB   # q blocks (4)
SUB = QB // KT    # k tiles per q block on the diagonal (4)


def build_program(use_kpm=False):
    """Emit the SPMD program (identical on all 8 cores)."""
    fp32 = mybir.dt.float32
    bf16 = mybir.dt.bfloat16

    nc = bacc.Bacc(num_devices=NCORES)

    xqT = nc.declare_dram_parameter("xqT", [D, SEQ], bf16, False)
    xkT = nc.declare_dram_parameter("xkT", [D, SEQ], bf16, False)
    xvT = nc.declare_dram_parameter("xvT", [D, SEQ], bf16, False)
    wqT = nc.declare_dram_parameter("wqT", [D, DPC], bf16, False)
    wkT = nc.declare_dram_parameter("wkT", [D, DPC], bf16, False)
    wvT = nc.declare_dram_parameter("wvT", [D, DPC], bf16, False)
    wvb = nc.declare_dram_parameter("wvb", [1, DPC], bf16, False)
    woT = nc.declare_dram_parameter("woT", [DPC, D], bf16, False)
    bq2_d = nc.declare_dram_parameter("bq2", [128, NMT], fp32, False)
    bk2_d = nc.declare_dram_parameter("bk2", [128, NMT], fp32, False)
    tri_d = nc.declare_dram_parameter("tri", [KT, KT], bf16, False)
    ident_d = nc.declare_dram_parameter("ident", [128, 128], bf16, False)
    bo2_d = nc.declare_dram_parameter("bo2", [128, D], fp32, False)
    kpm_d = (nc.declare_dram_parameter("kpmT", [128, SEQ // 128], fp32, False)
             if use_kpm else None)
    out_ext = nc.declare_dram_parameter("out", [SEQ // G, D], bf16, isOutput=True)

    partial_dram = nc.dram_tensor("partial", [SEQ, D], bf16)
    rs_out = nc.dram_tensor("rs_out", [SEQ // G, D], bf16)

    groups = [[0, 1, 2, 3], [4, 5, 6, 7]]

    with ExitStack() as ctx:
        tc = ctx.enter_context(tile.TileContext(nc, num_cores=NCORES))

        xpool = ctx.enter_context(tc.tile_pool(name="xp", bufs=28))
        persist = ctx.enter_context(tc.tile_pool(name="persist", bufs=1))
        ppool = ctx.enter_context(tc.tile_pool(name="pp", bufs=34))
        cqpool = ctx.enter_context(tc.tile_pool(name="cq", bufs=4))
        opool = ctx.enter_context(tc.tile_pool(name="op", bufs=4))
        spool = ctx.enter_context(tc.tile_pool(name="sp", bufs=8))
        psP = ctx.enter_context(tc.tile_pool(name="psP", bufs=2, space="PSUM"))
        psM = ctx.enter_context(tc.tile_pool(name="psM", bufs=2, space="PSUM"))
        psC = ctx.enter_context(tc.tile_pool(name="psC", bufs=2, space="PSUM"))

        # ---- small constants (spread over queues; all early) ----
        tri_sb = persist.tile([KT, KT], bf16, tag="tri")
        nc.scalar.dma_start(out=tri_sb, in_=tri_d[:, :])
        ident_sb = persist.tile([128, 128], bf16, tag="ident")
        nc.scalar.dma_start(out=ident_sb, in_=ident_d[:, :])
        bq2_sb = persist.tile([128, NMT], fp32, tag="bq2")
        nc.scalar.dma_start(out=bq2_sb, in_=bq2_d[:, :])
        bk2_sb = persist.tile([128, NMT], fp32, tag="bk2")
        nc.scalar.dma_start(out=bk2_sb, in_=bk2_d[:, :])
        bo2_sb = persist.tile([128, D], fp32, tag="bo2")
        nc.scalar.dma_start(out=bo2_sb, in_=bo2_d[:, :])
        wvb_sb = persist.tile([1, DPC], bf16, tag="wvb")
        nc.scalar.dma_start(out=wvb_sb, in_=wvb[0:1, :])
        ones1 = persist.tile([1, 128], bf16, tag="ones1")
        nc.vector.memset(ones1, 1.0)
        kpm_sb = None
        if use_kpm:
            kpm_sb = persist.tile([128, SEQ // 128], fp32, tag="kpm")
            nc.scalar.dma_start(out=kpm_sb, in_=kpm_d[:, :])
        wo_sb = [persist.tile([128, D], bf16, tag=f"wo{t}", name=f"wo{t}")
                 for t in range(NMT)]
        for t in range(NMT):
            nc.scalar.dma_start(out=wo_sb[t], in_=woT[t * 128:(t + 1) * 128, :])

        # ---- persistent weights (consumed every block; load once) ----
        wk_t = [persist.tile([128, DPC], bf16, tag=f"wk{k}", name=f"wk{k}")
                for k in range(NDT)]
        wq_t = [persist.tile([128, DPC], bf16, tag=f"wq{k}", name=f"wq{k}")
                for k in range(NDT)]
        wv_t = [persist.tile([128, DPC], bf16, tag=f"wv{k}", name=f"wv{k}")
                for k in range(NDT)]

        # ---- persistent activations ----
        # K2/Q2 are mt-major: rows = the 128 head dims of heads (2mt, 2mt+1).
        K2 = [persist.tile([128, SEQ], bf16, tag=f"K2{t}", name=f"K2{t}")
              for t in range(NMT)]
        Q2 = [persist.tile([128, SEQ], bf16, tag=f"Q2{t}", name=f"Q2{t}")
              for t in range(NMT)]
        V_sb = [persist.tile([128, HPC, 65], bf16, tag=f"V{m}", name=f"V{m}")
                for m in range(SEQ // KT)]
        ctxT = [persist.tile([128, SEQ], bf16, tag=f"ctxT{t}", name=f"ctxT{t}")
                for t in range(NMT)]

        def kq_dma(blk):
            """Issue K/Q input DMAs for q/k columns [blk*QB, (blk+1)*QB)."""
            c0 = blk * QB
            eng = nc.gpsimd if blk <= 1 else nc.sync
            xk_t, xq_t = [], []
            for k in range(NDT):
                xk = xpool.tile([128, QB], bf16, tag="xt", name=f"xk{blk}_{k}")
                nc.sync.dma_start(out=xk, in_=xkT[k * 128:(k + 1) * 128, c0:c0 + QB])
                if blk == 0:
                    nc.sync.dma_start(out=wk_t[k], in_=wkT[k * 128:(k + 1) * 128, :])
                xq = xpool.tile([128, QB], bf16, tag="xt", name=f"xq{blk}_{k}")
                eng.dma_start(out=xq, in_=xqT[k * 128:(k + 1) * 128, c0:c0 + QB])
                if blk == 0:
                    eng.dma_start(out=wq_t[k], in_=wqT[k * 128:(k + 1) * 128, :])
                xk_t.append(xk)
                xq_t.append(xq)
            return xk_t, xq_t

        def kq_gen(blk, mt, xk_t, xq_t):
            """K and Q projection matmuls for one mt half (yield/matmul).
            Splitting per-mt lets scores for heads 2mt/2mt+1 start after
            only this half's eviction."""
            c0 = blk * QB
            pskq = psP.tile([128, 2 * QB], fp32, tag="p2", name="pskq")
            psk, psq = pskq[:, 0:QB], pskq[:, QB:2 * QB]
            for k in range(NDT):
                st, sp = (k == 0), (k == NDT - 1)
                nc.tensor.matmul(out=psk, rhs=xk_t[k],
                                 lhsT=wk_t[k][:, mt * 128:(mt + 1) * 128],
                                 start=st, stop=sp)
                yield
                nc.tensor.matmul(out=psq, rhs=xq_t[k],
                                 lhsT=wq_t[k][:, mt * 128:(mt + 1) * 128],
                                 start=st, stop=sp)
                yield
            nc.vector.tensor_scalar(
                out=K2[mt][:, c0:c0 + QB], in0=psk,
                scalar1=bk2_sb[:, mt:mt + 1], scalar2=None,
                op0=mybir.AluOpType.add)
            nc.vector.tensor_scalar(
                out=Q2[mt][:, c0:c0 + QB], in0=psq,
                scalar1=bq2_sb[:, mt:mt + 1], scalar2=None,
                op0=mybir.AluOpType.add)

        def v_dma(blk):
            c0 = blk * QB
            eng = nc.gpsimd if blk <= 1 else nc.sync
            xv_t = []
            for k in range(NDT):
                xv = xpool.tile([128, QB], bf16, tag="xt", name=f"xv{blk}_{k}")
                eng.dma_start(out=xv, in_=xvT[k * 128:(k + 1) * 128, c0:c0 + QB])
                if blk == 0:
                    eng.dma_start(out=wv_t[k], in_=wvT[k * 128:(k + 1) * 128, :])
                xv_t.append(xv)
            return xv_t

        def v_gen(blk, xv_t):
            """V projection for tokens [blk*QB, ...) -> V_sb[4blk..4blk+3]."""
            for w in range(2):
                psv = psP.tile([128, 2 * QB], fp32, tag="p2", name="psv")
                ps = [psv[:, 0:DPC], psv[:, QB:QB + DPC]]
                for k in range(NDT):
                    for i in range(2):
                        m2 = 2 * w + i
                        nc.tensor.matmul(out=ps[i], rhs=wv_t[k],
                                         lhsT=xv_t[k][:, m2 * 128:(m2 + 1) * 128],
                                         start=(k == 0), stop=False)
                        yield
                for i in range(2):
                    mt = SUB * blk + 2 * w + i
                    nc.tensor.matmul(out=ps[i], rhs=wvb_sb[0:1, :],
                                     lhsT=ones1[0:1, :], start=False, stop=True)
                    nc.vector.tensor_copy(
                        out=V_sb[mt][:, :, 0:64],
                        in_=ps[i].rearrange("p (h e) -> p h e", h=HPC))
                    nc.vector.memset(V_sb[mt][:, :, 64:65], 1.0)
                    yield

        def sc_head(qb, h, pts, pump):
            """Scores + exp + causal mask for one head of q block qb."""
            q0 = qb * QB
            mt, hh = divmod(h, 2)
            krows = slice(64 * hh, 64 * hh + 64)
            nfull = SUB * qb
            for i in range(nfull // 2):
                # two full k tiles share a 2-bank PSUM tile and one wide exp
                k0 = 2 * i
                s2 = psP.tile([128, 2 * QB], fp32, tag="p2", name=f"s2{h}_{i}")
                pt2 = ppool.tile([128, 2 * QB], bf16, tag="pt2",
                                 name=f"pt2{h}_{i}")
                for d in range(2):
                    nc.tensor.matmul(
                        out=s2[:, d * QB:(d + 1) * QB],
                        lhsT=K2[mt][krows, (k0 + d) * KT:(k0 + d + 1) * KT],
                        rhs=Q2[mt][krows, q0:q0 + QB],
                        start=True, stop=True)
                    pump(1)
                nc.scalar.activation(
                    out=pt2, in_=s2,
                    func=mybir.ActivationFunctionType.Exp)
                if use_kpm:
                    for d in range(2):
                        nc.vector.tensor_scalar(
                            out=pt2[:, d * QB:(d + 1) * QB],
                            in0=pt2[:, d * QB:(d + 1) * QB],
                            scalar1=kpm_sb[:, k0 + d:k0 + d + 1], scalar2=None,
                            op0=mybir.AluOpType.mult)
                pts[h, k0] = pt2[:, 0:QB]
                pts[h, k0 + 1] = pt2[:, QB:2 * QB]
                pump(3)
            for kti in range(nfull, nfull + SUB):
                o = 128 * (kti - nfull)
                s_ps = psM.tile([128, QB], fp32, tag="m", name=f"s{h}_{kti}")
                nc.tensor.matmul(
                    out=s_ps[:, o:QB],
                    lhsT=K2[mt][krows, kti * KT:(kti + 1) * KT],
                    rhs=Q2[mt][krows, q0 + o:q0 + QB],
                    start=True, stop=True)
                pt = ppool.tile([128, QB], bf16, tag="pt", name=f"pt{h}_{kti}")
                nc.scalar.activation(
                    out=pt[:, o:QB], in_=s_ps[:, o:QB],
                    func=mybir.ActivationFunctionType.Exp)
                nc.vector.tensor_mul(
                    out=pt[:, o:o + KT], in0=pt[:, o:o + KT], in1=tri_sb)
                if use_kpm:
                    nc.vector.tensor_scalar(
                        out=pt[:, o:QB], in0=pt[:, o:QB],
                        scalar1=kpm_sb[:, kti:kti + 1], scalar2=None,
                        op0=mybir.AluOpType.mult)
                pts[h, kti] = pt
                pump(3)

        def pv_gen(qb, p, pts):
            """PV for head pair p; ctx lands q-major; one transpose/chunk."""
            q0 = qb * QB
            h0, h1 = 2 * p, 2 * p + 1

            def flush(qs, cq):
                tr_ps = psC.tile([128, 128], bf16, tag="c")
                nc.tensor.transpose(out=tr_ps, in_=cq, identity=ident_sb)
                nc.vector.tensor_copy(
                    out=ctxT[p][:, q0 + qs * 128:q0 + (qs + 1) * 128],
                    in_=tr_ps)

            pend = None
            for qs in range(SUB):
                cq = cqpool.tile([128, 128], bf16, tag="cq")
                for j, h in enumerate((h0, h1)):
                    ctx_ps = psC.tile([128, 65], fp32, tag="c", name=f"ctx{j}")
                    for kti in range(SUB * qb + qs + 1):
                        nc.tensor.matmul(
                            out=ctx_ps,
                            lhsT=pts[h, kti][:, qs * 128:(qs + 1) * 128],
                            rhs=V_sb[kti][:, h, :],
                            start=(kti == 0), stop=(kti == SUB * qb + qs))
                    rcp = spool.tile([128, 1], fp32, tag="rcp")
                    nc.vector.reciprocal(out=rcp, in_=ctx_ps[:, 64:65])
                    nc.vector.tensor_scalar(
                        out=cq[:, 64 * j:64 * j + 64], in0=ctx_ps[:, 0:64],
                        scalar1=rcp, scalar2=None, op0=mybir.AluOpType.mult)
                    yield
                if pend is not None:
                    flush(*pend)
                pend = (qs, cq)
            flush(*pend)

        def op_gen(qb):
            """Out-proj rows [qb*QB, ...) -> partial -> this chunk's RS."""
            for mt in range(SUB * qb, SUB * qb + SUB):
                po = opool.tile([128, D], bf16, tag="po")
                for oc in range(D // QB):
                    ps = psM.tile([128, QB], fp32, tag="m", name="pso")
                    for t in range(NMT):
                        nc.tensor.matmul(
                            out=ps,
                            lhsT=ctxT[t][:, mt * 128:(mt + 1) * 128],
                            rhs=wo_sb[t][:, oc * QB:(oc + 1) * QB],
                            start=(t == 0), stop=(t == NMT - 1))
                        yield
                    nc.vector.tensor_add(
                        out=po[:, oc * QB:(oc + 1) * QB],
                        in0=ps, in1=bo2_sb[:, oc * QB:(oc + 1) * QB])
                nc.sync.dma_start(
                    out=partial_dram[mt * 128:(mt + 1) * 128, :], in_=po)
            # chunks qb0/qb1 ReduceScatter alone (early, hidden under
            # compute); qb2+qb3 merge into one collective fired after qb3 —
            # the two tail chunks land nearly together, so one 28us RS beats
            # two serialized 21.5us ones.
            if qb <= 1:
                nc.gpsimd.collective_compute(
                    "ReduceScatter",
                    mybir.AluOpType.add,
                    replica_groups=groups,
                    ins=[partial_dram[qb * QB:(qb + 1) * QB, :]],
                    outs=[rs_out[qb * 128:(qb + 1) * 128, :]],
                )
            elif qb == 3:
                nc.gpsimd.collective_compute(
                    "ReduceScatter",
                    mybir.AluOpType.add,
                    replica_groups=groups,
                    ins=[partial_dram[2 * QB:4 * QB, :]],
                    outs=[rs_out[256:512, :]],
                )

        # Weaved emission: score tiles are exp-paced (the scalar engine
        # drains one [128,512] exp per ~610ns vs the 213ns score matmul), so
        # after each score tile we pump ~2 steps of queued exp-free work
        # (projections, PV, out-proj+RS) into the PE stream.  Queue is FIFO
        # in schedule order, so data dependencies (V before PV, PV before
        # out-proj) drain in order; kq blocks a score head reads are force-
        # drained first so readers never precede their writers.
        sched = [
            ("kq", 0),
            ("sc", 0, 0), ("sc", 0, 1),
            ("v", 0), ("kq", 1),
            ("sc", 0, 2), ("sc", 0, 3),
            ("kq", 2), ("v", 1),
            ("sc", 1, 0), ("sc", 1, 1), ("pv", 0, 0),
            ("sc", 1, 2), ("sc", 1, 3), ("pv", 0, 1),
            ("op", 0),
            ("kq", 3), ("v", 2),
            ("sc", 2, 0), ("sc", 2, 1), ("pv", 1, 0),
            ("sc", 2, 2), ("sc", 2, 3), ("pv", 1, 1),
            ("op", 1),
            ("v", 3),
            ("sc", 3, 0), ("sc", 3, 1), ("pv", 2, 0),
            ("sc", 3, 2), ("pv", 2, 1),
            ("op", 2),
            ("sc", 3, 3),
            ("pv", 3, 0), ("pv", 3, 1),
            ("op", 3),
        ]
        pts_all = {qb: {} for qb in range(NQB)}
        queue = deque()

        def pump(n):
            while n > 0 and queue:
                try:
                    next(queue[0][1])
                    n -= 1
                except StopIteration:
                    queue.popleft()

        def drain_through(pred):
            """Fully drain queue head through the last unit matching pred."""
            if not any(pred(key) for key, _ in queue):
                return
            last = max(i for i, (key, _) in enumerate(queue) if pred(key))
            for _ in range(last + 1):
                key, gen = queue.popleft()
                for _ in gen:
                    pass

        for unit in sched:
            kind = unit[0]
            if kind == "kq":
                xk_t, xq_t = kq_dma(unit[1])
                for mt in range(NMT):
                    queue.append((("kq", unit[1], mt),
                                  kq_gen(unit[1], mt, xk_t, xq_t)))
            elif kind == "v":
                xv_t = v_dma(unit[1])
                queue.append((unit, v_gen(unit[1], xv_t)))
            elif kind == "pv":
                queue.append((unit, pv_gen(unit[1], unit[2], pts_all[unit[1]])))
            elif kind == "op":
                queue.append((unit, op_gen(unit[1])))
            else:
                qb, h = unit[1], unit[2]
                drain_through(lambda key: key[0] == "kq" and key[1] <= qb
                              and key[2] <= h // 2)
                sc_head(qb, h, pts_all[qb], pump)
        while queue:
            pump(1000)

        # rs_out -> out via SBUF bounces.  Chunks 0/1 ride the idle sync
        # queue and hide under later compute; the tail-exposed merged chunk
        # is split across the sync and scalar queues so its two row tiles
        # bounce in parallel.
        for qb in range(2):
            osb = opool.tile([128, D], bf16, tag="osb")
            nc.sync.dma_start(out=osb, in_=rs_out[qb * 128:(qb + 1) * 128, :])
            nc.sync.dma_start(out=out_ext[qb * 128:(qb + 1) * 128, :], in_=osb)
        for i, eng in enumerate((nc.sync, nc.scalar)):
            r0 = 256 + 128 * i
            osb = opool.tile([128, D], bf16, tag="osb")
            eng.dma_start(out=osb, in_=rs_out[r0:r0 + 128, :])
            eng.dma_start(out=out_ext[r0:r0 + 128, :], in_=osb)

    nc.compile()
    return nc


def prep_core_inputs(inputs, core):
    """Host-side shard/layout prep for one core.  Pure layout + dtype work."""
    b, g = divmod(core, G)
    sl = slice(g * DPC, (g + 1) * DPC)
    s = 1.0 / np.sqrt(DH)

    def xT(x):
        return np.ascontiguousarray(np.asarray(x)[b].T).astype(BF16)

    def wT(w, scale=1.0):
        wt = np.asarray(w)[sl, :].T.astype(np.float32)
        return (wt * scale).astype(BF16)

    def b2(bias, scale=1.0):
        bb = (np.asarray(bias)[sl].astype(np.float32) * scale)
        return np.ascontiguousarray(bb.reshape(NMT, 128).T)

    kpm = np.asarray(inputs["key_padding_mask"])
    tri = (np.arange(KT)[:, None] <= np.arange(KT)[None, :]).astype(BF16)

    d = {
        "xqT": xT(inputs["q_input"]),
        "xkT": xT(inputs["k_input"]),
        "xvT": xT(inputs["v_input"]),
        "wqT": wT(inputs["wq"], s),
        "wkT": wT(inputs["wk"]),
        "wvT": wT(inputs["wv"]),
        "wvb": np.asarray(inputs["bv"])[sl].astype(BF16)[None, :],
        "woT": np.ascontiguousarray(np.asarray(inputs["wo"]).T[sl, :]).astype(BF16),
        "bq2": b2(inputs["bq"], s),
        "bk2": b2(inputs["bk"]),
        "tri": tri,
        "ident": np.eye(128, dtype=BF16),
        "bo2": np.tile(np.asarray(inputs["bo"]).astype(np.float32) / G, (128, 1)),
    }
    if kpm.any():
        # kpmT[p, kti] = 0.0 for padded key (128*kti + p), else 1.0
        d["kpmT"] = np.ascontiguousarray(
            1.0 - kpm[b].astype(np.float32).reshape(SEQ // 128, 128).T)
    return d


def assemble_output(core_outs):
    """core 4b+j: chunks qb0/qb1 hold reduced rows [512qb+128j, +128);
    the merged qb2+qb3 chunk holds rows [1024+256j, +256)."""
    out = np.empty((B, SEQ, D), dtype=np.float32)
    for core in range(NCORES):
        b, j = divmod(core, G)
        co = np.asarray(core_outs[core]).astype(np.float32)
        for qb in range(2):
            r0 = QB * qb + 128 * j
            out[b, r0:r0 + 128, :] = co[qb * 128:(qb + 1) * 128]
        r0 = 2 * QB + 256 * j
        out[b, r0:r0 + 256, :] = co[256:512]
    return out


_CACHED = {}


def _get_nc(use_kpm=False):
    if use_kpm not in _CACHED:
        _CACHED[use_kpm] = build_program(use_kpm=use_kpm)
    return _CACHED[use_kpm]


def kernel(**inputs) -> np.ndarray:
    use_kpm = bool(np.asarray(inputs["key_padding_mask"]).any())
    nc = _get_nc(use_kpm)
    in_maps = [prep_core_inputs(inputs, core) for core in range(NCORES)]
    res = run_bass_kernel_spmd(nc, in_maps, core_ids=list(range(NCORES)))
    return assemble_output([res.results[c]["out"] for c in range(NCORES)])


if __name__ == "__main__":
    nc = build_program()
    print("program built ok")

